# revision 7
# baseline (speedup 1.0000x reference)
"""Exact KNN collision kernel for trn2 (8 NeuronCores).

Computes nn[b,n] = argmin_m |vertices[b,n] - collider[b, cvi[m]]|^2 with the
reference's exact fp32 arithmetic and first-occurrence tie-breaking.

Strategy per core (core c -> batch b=c//2, row-half h=c%2, 8192 rows):
  - host dedups gathered collider points (order preserved by first occurrence)
  - PE: dot = v @ cv^T  (K=3 fp32 matmul, 128-row x 512-col chunks -> PSUM)
  - DVE: s = dot - c2/2 (bitwise == -d2/2 of the reference => same argmin/ties)
  - DVE: rowmax = reduce_max(s); idx = max_index(s, rowmax) (first match)
  - host maps dedup slot -> first position in collision_vertices
"""
import os
import sys
import numpy as np

_BASS_PATH = "/opt/trn_rl_repo"
if _BASS_PATH not in sys.path:
    sys.path.insert(0, _BASS_PATH)

B, N, V, M = 4, 16384, 6890, 4096
NCORES = 8
ROWS = (B * N) // NCORES          # 8192 rows per core
NT = ROWS // 128                  # 64 row tiles
VARIANT = os.environ.get("KNN_VARIANT", "native3")

_PROGRAM_CACHE = {}


def _build_program(U):
    import concourse.bacc as bacc
    import concourse.mybir as mybir
    import concourse.tile as tile

    f32 = mybir.dt.float32
    u32 = mybir.dt.uint32
    MP = ((U + 511) // 512) * 512

    nc = bacc.Bacc("TRN2", target_bir_lowering=False, debug=False, num_devices=NCORES)
    vc = nc.dram_tensor("vc", [3, ROWS + MP], f32, kind="ExternalInput")
    c2h = nc.dram_tensor("c2h", [128, MP], f32, kind="ExternalInput")
    out = nc.dram_tensor("idx", [NT, 128, 1], u32, kind="ExternalOutput")

    # matmul chunk column ranges covering exactly [0, U)
    chunks = []
    j0 = 0
    while j0 < U:
        chunks.append((j0, min(j0 + 512, U)))
        j0 += 512

    with tile.TileContext(nc) as tc:
        with (
            tc.tile_pool(name="const", bufs=1) as cpool,
            tc.tile_pool(name="work", bufs=2) as wpool,
            tc.tile_pool(name="psum", bufs=1, space="PSUM") as ppool,
        ):
            vc_sb = cpool.tile([3, ROWS + MP], f32)
            c2h_sb = cpool.tile([128, MP], f32)
            nc.sync.dma_start(vc_sb[:], vc[:])
            for q in range(4):
                nc.sync.dma_start(c2h_sb[q * 32:(q + 1) * 32, :],
                                  c2h[q * 32:(q + 1) * 32, :])

            for t in range(NT):
                dot = ppool.tile([128, MP], f32, tag="dot")
                for (a, b) in chunks:
                    nc.tensor.matmul(
                        dot[:, a:b],
                        vc_sb[:, t * 128:(t + 1) * 128],
                        vc_sb[:, ROWS + a:ROWS + b],
                        start=True, stop=True,
                    )
                s_sb = wpool.tile([128, U], f32, tag="s")
                rowmax = wpool.tile([128, 1], f32, tag="rm")
                idx8 = wpool.tile([128, 8], u32, tag="idx")
                nc.vector.tensor_sub(s_sb[:], dot[:, :U], c2h_sb[:, :U])
                nc.vector.tensor_reduce(
                    rowmax[:], s_sb[:], axis=mybir.AxisListType.X,
                    op=mybir.AluOpType.max)
                nc.vector.max_index(
                    idx8[:], rowmax[:, 0:1].to_broadcast((128, 8)), s_sb[:])
                nc.sync.dma_start(out[t], idx8[:, 0:1])
    nc.compile()
    return nc


def _get_program(MP):
    if MP not in _PROGRAM_CACHE:
        _PROGRAM_CACHE[MP] = _build_program(MP)
    return _PROGRAM_CACHE[MP]


def kernel(vertices, collider, collision_vertices, _want_trace=False):
    from concourse.bass_utils import run_bass_kernel_spmd

    v = np.ascontiguousarray(np.asarray(vertices), dtype=np.float32)     # [B,N,3]
    c = np.ascontiguousarray(np.asarray(collider), dtype=np.float32)     # [B,V,3]
    cvi = np.asarray(collision_vertices).astype(np.int64)                # [M]

    # dedup candidates, keeping first-occurrence order (exact tie semantics)
    u, first_pos = np.unique(cvi, return_index=True)
    order = np.argsort(first_pos)
    u = u[order]
    first_pos = first_pos[order].astype(np.int32)
    U = len(u)
    MP = ((U + 511) // 512) * 512

    cv = c[:, u, :]                                            # [B,U,3]
    c2h = (cv * cv).sum(-1, dtype=np.float32) * np.float32(0.5)  # [B,U]

    cvT_pad = np.zeros((B, 3, MP), np.float32)
    cvT_pad[:, :, :U] = cv.transpose(0, 2, 1)
    c2h_pad = np.full((B, MP), np.float32(5e29), np.float32)
    c2h_pad[:, :U] = c2h

    in_maps = []
    for core in range(NCORES):
        b = core // 2
        r0 = (core % 2) * ROWS
        vT = v[b, r0:r0 + ROWS, :].T                           # [3, ROWS]
        in_maps.append({
            "vc": np.ascontiguousarray(
                np.concatenate([vT, cvT_pad[b]], axis=1), dtype=np.float32),
            "c2h": np.ascontiguousarray(
                np.broadcast_to(c2h_pad[b][None, :], (128, MP)), dtype=np.float32),
        })

    nc = _get_program(U)
    res = run_bass_kernel_spmd(nc, in_maps, core_ids=list(range(NCORES)))

    nn = np.zeros((B, N), np.int32)
    for core in range(NCORES):
        b = core // 2
        r0 = (core % 2) * ROWS
        k = res.results[core]["idx"].reshape(-1).astype(np.int64)  # slot in dedup space
        nn[b, r0:r0 + ROWS] = first_pos[k]
    batch_idx = np.broadcast_to(np.arange(B, dtype=np.int32)[:, None], nn.shape)
    outv = np.stack([batch_idx, nn], axis=-1).astype(np.int32)
    if _want_trace:
        return outv, (res, in_maps)
    return outv


# revision 9
# speedup vs baseline: 1.6214x; 1.6214x over previous
"""Exact KNN collision kernel for trn2 (8 NeuronCores).

Computes nn[b,n] = argmin_m |vertices[b,n] - collider[b, cvi[m]]|^2 with the
reference's exact fp32 arithmetic and first-occurrence tie-breaking.

Strategy per core (core c -> batch b=c//2, row-half h=c%2, 8192 rows):
  - host dedups gathered collider points (order preserved by first occurrence)
  - PE: dot = v @ cv^T  (K=3 fp32 matmul, 128-row x 512-col chunks -> PSUM)
  - DVE: s = dot - c2/2 (bitwise == -d2/2 of the reference => same argmin/ties)
  - DVE: rowmax = reduce_max(s); idx = max_index(s, rowmax) (first match)
  - host maps dedup slot -> first position in collision_vertices
"""
import os
import sys
import numpy as np

_BASS_PATH = "/opt/trn_rl_repo"
if _BASS_PATH not in sys.path:
    sys.path.insert(0, _BASS_PATH)

B, N, V, M = 4, 16384, 6890, 4096
NCORES = 8
ROWS = (B * N) // NCORES          # 8192 rows per core
NT = ROWS // 128                  # 64 row tiles
VARIANT = os.environ.get("KNN_VARIANT", "fused")

_PROGRAM_CACHE = {}


def _register_sub_max():
    """Register a custom DVE op: out = in0 - in1; accum_out = max(s0, max(out)).

    Fuses the c2/2 subtraction with the row-max reduction in one Vector pass
    (the stock TENSOR_TENSOR_REDUCE ISA op is broken on this runtime).
    """
    from concourse import dve_ops
    from concourse.dve_spec import Spec, Src0, Src1, C0, maxx, lower
    from concourse.dve_spec import _has_src1
    from concourse.dve_uop import DveOpSpec

    name = "SUB_MAX_REDUCE_ANT"
    if name in dve_ops._SUB_OPCODE_FOR_NAME:
        return dve_ops._SUB_MAX_REDUCE_ANT

    def _ref(in0, in1, c0, c1, c2):
        body = (np.asarray(in0, np.float32) - np.asarray(in1, np.float32)).astype(np.float32)
        seed = np.asarray(c0, np.float32).reshape(-1, 1)
        acc = np.maximum(np.maximum.reduce(body.reshape(body.shape[0], -1),
                                           axis=-1, keepdims=True), seed)
        return body, acc

    spec = Spec(body=Src0 - Src1, accum=maxx, accum_init=C0, reference=_ref)
    shas = {}
    for ver in ("v3", "v4"):
        tmp = DveOpSpec(name=name, opcode=31, uops=lower(spec, ver=ver),
                        rd1_en=_has_src1(spec))
        shas[ver] = tmp.sha(ver)
    op = dve_ops.DveOp(name, spec, subdim=False, uops_sha=shas)
    row = max(dve_ops._SUB_OPCODE_FOR_NAME.values()) + 1
    assert row < 0x20
    dve_ops.OPS.append(op)
    dve_ops.CUSTOM_DVE_SPECS[name] = spec
    dve_ops._SUB_OPCODE_FOR_NAME[name] = row
    dve_ops._SUB_MAX_REDUCE_ANT = op
    return op


def _build_program(U):
    import concourse.bacc as bacc
    import concourse.mybir as mybir
    import concourse.tile as tile

    f32 = mybir.dt.float32
    u32 = mybir.dt.uint32
    MP = ((U + 511) // 512) * 512

    nc = bacc.Bacc("TRN2", target_bir_lowering=False, debug=False, num_devices=NCORES)
    vc = nc.dram_tensor("vc", [3, ROWS + MP], f32, kind="ExternalInput")
    c2h = nc.dram_tensor("c2h", [128, MP], f32, kind="ExternalInput")
    out = nc.dram_tensor("idx", [NT, 128, 1], u32, kind="ExternalOutput")

    # matmul chunk column ranges covering exactly [0, U)
    chunks = []
    j0 = 0
    while j0 < U:
        chunks.append((j0, min(j0 + 512, U)))
        j0 += 512

    with tile.TileContext(nc) as tc:
        with (
            tc.tile_pool(name="const", bufs=1) as cpool,
            tc.tile_pool(name="work", bufs=2) as wpool,
            tc.tile_pool(name="psum", bufs=1, space="PSUM") as ppool,
        ):
            vc_sb = cpool.tile([3, ROWS + MP], f32)
            c2h_sb = cpool.tile([128, MP], f32)
            nc.sync.dma_start(vc_sb[:], vc[:])
            for q in range(4):
                nc.sync.dma_start(c2h_sb[q * 32:(q + 1) * 32, :],
                                  c2h[q * 32:(q + 1) * 32, :])

            SPLIT = 2048 if U > 2048 else 512
            UB = U - SPLIT
            subop = _register_sub_max() if VARIANT == "fused" else None
            for t in range(NT):
                if VARIANT == "fused":
                    dotA = ppool.tile([128, SPLIT], f32, tag="dotA")
                    dotB = ppool.tile([128, MP - SPLIT], f32, tag="dotB")
                    for (a, b) in chunks:
                        dst = dotA[:, a:b] if b <= SPLIT else dotB[:, a - SPLIT:b - SPLIT]
                        nc.tensor.matmul(
                            dst,
                            vc_sb[:, t * 128:(t + 1) * 128],
                            vc_sb[:, ROWS + a:ROWS + b],
                            start=True, stop=True,
                        )
                    dcp = wpool.tile([128, U], f32, tag="dcp")
                    nc.scalar.copy(dcp[:, :SPLIT], dotA[:])
                    nc.scalar.copy(dcp[:, SPLIT:U], dotB[:, :UB])
                    s_sb = wpool.tile([128, U], f32, tag="s")
                    rowmax = wpool.tile([128, 1], f32, tag="rm")
                    idx8 = wpool.tile([128, 8], u32, tag="idx")
                    nc.vector._custom_dve(
                        subop, out=s_sb[:], in0=dcp[:], in1=c2h_sb[:, :U],
                        s0=-3.4e38, accum_out=rowmax[:])
                    nc.vector.max_index(
                        idx8[:], rowmax[:, 0:1].to_broadcast((128, 8)), s_sb[:])
                else:
                    dot = ppool.tile([128, MP], f32, tag="dot")
                    for (a, b) in chunks:
                        nc.tensor.matmul(
                            dot[:, a:b],
                            vc_sb[:, t * 128:(t + 1) * 128],
                            vc_sb[:, ROWS + a:ROWS + b],
                            start=True, stop=True,
                        )
                    s_sb = wpool.tile([128, U], f32, tag="s")
                    rowmax = wpool.tile([128, 1], f32, tag="rm")
                    idx8 = wpool.tile([128, 8], u32, tag="idx")
                    nc.vector.tensor_sub(s_sb[:], dot[:, :U], c2h_sb[:, :U])
                    nc.vector.tensor_reduce(
                        rowmax[:], s_sb[:], axis=mybir.AxisListType.X,
                        op=mybir.AluOpType.max)
                    nc.vector.max_index(
                        idx8[:], rowmax[:, 0:1].to_broadcast((128, 8)), s_sb[:])
                nc.sync.dma_start(out[t], idx8[:, 0:1])
    nc.compile()
    return nc


def _get_program(MP):
    if MP not in _PROGRAM_CACHE:
        _PROGRAM_CACHE[MP] = _build_program(MP)
    return _PROGRAM_CACHE[MP]


def kernel(vertices, collider, collision_vertices, _want_trace=False):
    from concourse.bass_utils import run_bass_kernel_spmd

    v = np.ascontiguousarray(np.asarray(vertices), dtype=np.float32)     # [B,N,3]
    c = np.ascontiguousarray(np.asarray(collider), dtype=np.float32)     # [B,V,3]
    cvi = np.asarray(collision_vertices).astype(np.int64)                # [M]

    # dedup candidates, keeping first-occurrence order (exact tie semantics)
    u, first_pos = np.unique(cvi, return_index=True)
    order = np.argsort(first_pos)
    u = u[order]
    first_pos = first_pos[order].astype(np.int32)
    U = len(u)
    MP = ((U + 511) // 512) * 512

    cv = c[:, u, :]                                            # [B,U,3]
    c2h = (cv * cv).sum(-1, dtype=np.float32) * np.float32(0.5)  # [B,U]

    cvT_pad = np.zeros((B, 3, MP), np.float32)
    cvT_pad[:, :, :U] = cv.transpose(0, 2, 1)
    c2h_pad = np.full((B, MP), np.float32(5e29), np.float32)
    c2h_pad[:, :U] = c2h

    in_maps = []
    for core in range(NCORES):
        b = core // 2
        r0 = (core % 2) * ROWS
        vT = v[b, r0:r0 + ROWS, :].T                           # [3, ROWS]
        in_maps.append({
            "vc": np.ascontiguousarray(
                np.concatenate([vT, cvT_pad[b]], axis=1), dtype=np.float32),
            "c2h": np.ascontiguousarray(
                np.broadcast_to(c2h_pad[b][None, :], (128, MP)), dtype=np.float32),
        })

    nc = _get_program(U)
    res = run_bass_kernel_spmd(nc, in_maps, core_ids=list(range(NCORES)))

    nn = np.zeros((B, N), np.int32)
    for core in range(NCORES):
        b = core // 2
        r0 = (core % 2) * ROWS
        k = res.results[core]["idx"].reshape(-1).astype(np.int64)  # slot in dedup space
        nn[b, r0:r0 + ROWS] = first_pos[k]
    batch_idx = np.broadcast_to(np.arange(B, dtype=np.int32)[:, None], nn.shape)
    outv = np.stack([batch_idx, nn], axis=-1).astype(np.int32)
    if _want_trace:
        return outv, (res, in_maps)
    return outv
